# revision 2
# baseline (speedup 1.0000x reference)
"""Trainium2 Bass kernel for relational GNN message passing (BlockDecomposition).

Math (per reference): directed edges (both directions of each input edge)
carry messages m_e = x[src_e] @ blockdiag(blocks[rel_e]); out[t] = sum_e
w_e * m_e over edges with tgt_e == t.

Strategy (8 NeuronCores, SPMD, shared instruction stream):
  - Partition output nodes across cores (12500 each). Each core processes
    exactly the directed edges whose target it owns -> no collective.
  - Per core, targets are split into 8 windows. Edges sorted by
    (window, relation).
  - Pass A per window: gather unique source rows (bf16) from DRAM into an
    SBUF "compact table" (dma_gather, int16 indices via 4 source ranges),
    then SBUF-source transposed dma_gather produces xT slabs (feature x
    edge). PE matmul with the dense 128x128 block-diagonal W_r (resident
    in SBUF) yields message tiles (edge x feature), stored contiguously to
    a DRAM msgs buffer in bf16.
  - Pass B per window: dma_gather message rows by target-sorted rank
    (indices are window-local -> int16 ok), build selection matrix
    S[e,t] = w_e * (tgt_e == t) on DVE in one tensor_scalar op, and
    matmul-accumulate S.T @ msgs into a PSUM tile per 128-target block.
    Each output row is written exactly once (no read-modify-write).
All data-dependent sizes are baked into the instruction stream at compile
time, padded to the max across the 8 cores so one program serves all.
"""

import os
import sys

sys.path.insert(0, "/opt/trn_rl_repo")

import numpy as np
import ml_dtypes

_PATCHED = False


def _patch_tile_drain():
    """This container's walrus accepts at most one sync-wait per instruction,
    but TileContext's kernel-tail attaches every outstanding DMA-lane wait to
    a single Drain ("Too many sync wait commands"). Spread the waits across
    individual SP NOPs before the drain."""
    global _PATCHED
    if _PATCHED:
        return
    _PATCHED = True
    import concourse.mybir as mybir
    import concourse.tile as tile_mod
    from bass_rust import ScopedClock

    def _drain_and_barrier(self, tick_clock, wait_clock):
        nc = self.nc
        collector = nc.sync.nop(nofuse=True, hint="drain_waits")
        wait_clock.add_sem_waits(
            collector.ins, ScopedClock({None: tick_clock.global_clock})
        )
        si = collector.ins.sync_info
        waits = list(si.on_wait) if si and si.on_wait else []
        if len(waits) > 1:
            si.on_wait = waits[:1]
            for wv in waits[1:]:
                n2 = nc.sync.nop(nofuse=True, hint="drain_waits")
                n2.ins.sync_info = mybir.SyncInfo(on_wait=[wv], on_update=[])
        nc.sync.drain()
        nc.all_engine_barrier()
        assert self.sems is not None
        popped = nc._tile_sem_poison_stack.pop()
        assert popped is self._sem_poison
        nc.clear_and_free_semaphores(list(self.sems.allocated().values()))
        nc.all_engine_barrier()

    tile_mod.TileContext._drain_and_barrier = _drain_and_barrier

# ---------------- problem constants (hardcoded) ----------------
N_NODES = 100000
D = 128
R = 64           # relations used by edges (blocks table has R+1 rows)
NB = 8
BS = 16
N_EDGES = 500000
NCORES = 8
P = 128
NT = N_NODES // NCORES          # 12500 targets per core
TBLOCKS = (NT + P - 1) // P     # 98
NT_PAD = TBLOCKS * P            # 12544
N_WIN = 8
SRC_RANGES = 4                  # int16 gather index ranges over N_NODES
RANGE_W = 25000                 # range width (< 32768)
GOP = 8192                      # gather op size (edges / msgs)

TRACE = os.environ.get("GNN_TRACE", "0") == "1"

bf16 = ml_dtypes.bfloat16

# window split of the 98 target blocks: two windows of 13, six of 12
_WSIZES = [13, 13, 12, 12, 12, 12, 12, 12]
assert sum(_WSIZES) == TBLOCKS
WIN_TB_START = np.cumsum([0] + _WSIZES)      # per-window first tblock
WIN_OF_TB = np.repeat(np.arange(N_WIN), _WSIZES)


def _idx_image(lst):
    """int16 index list (len % 128 == 0) -> SBUF image [128, len//16],
    entry i at (i%16, i//16), replicated across the 8 16-partition bands."""
    lst = np.asarray(lst, dtype=np.int16)
    n = len(lst)
    assert n % 128 == 0 and n > 0
    a = lst.reshape(n // 16, 16).T          # [16, n//16]
    return np.tile(a, (8, 1))               # [128, n//16]


def _ceil(a, b):
    return -(-a // b)


def _round_up(a, b):
    return _ceil(a, b) * b


def _preprocess(x, blocks, edge_weights, source, target, edge_type):
    """Host-side: build all per-core device inputs + the shared structure."""
    src = np.asarray(source).astype(np.int64)
    tgt = np.asarray(target).astype(np.int64)
    rel = np.asarray(edge_type).astype(np.int64)
    w = np.asarray(edge_weights).astype(np.float32)

    # directed edges (both directions, same relation/weight)
    s2 = np.concatenate([src, tgt])
    t2 = np.concatenate([tgt, src])
    r2 = np.concatenate([rel, rel])
    w2 = np.concatenate([w, w])

    owner = t2 // NT
    tloc = t2 - owner * NT
    tb = tloc // P
    win = WIN_OF_TB[tb]

    # ---- per-core edge lists sorted by (window, relation) ----
    cores = []
    for c in range(NCORES):
        m = owner == c
        key = win[m] * R + r2[m]
        order = np.argsort(key, kind="stable")
        cores.append({
            "s": s2[m][order],
            "tloc": tloc[m][order],
            "r": r2[m][order],
            "w": w2[m][order],
            "key": key[order],
        })

    # group sizes n[c, w*R+r]; shared caps
    NG = N_WIN * R
    n_grp = np.zeros((NCORES, NG), np.int64)
    for c in range(NCORES):
        n_grp[c] = np.bincount(cores[c]["key"], minlength=NG)
    cap = n_grp.max(axis=0)                       # shared group capacity
    grp_off = np.concatenate([[0], np.cumsum(cap)])  # msg position of group g
    total_msgs = int(grp_off[-1])

    # per-window msg base / extent
    MO = [int(grp_off[wv * R]) for wv in range(N_WIN)]
    MO.append(total_msgs)
    EW = [MO[wv + 1] - MO[wv] for wv in range(N_WIN)]
    for wv in range(N_WIN):
        assert EW[wv] < 32000, f"window {wv} msgs {EW[wv]} exceeds int16"

    # ---- compact source tables per (core, window); shared segment caps ----
    # uniq sources per (core, win, range) counts
    seg_cnt = np.zeros((NCORES, N_WIN, SRC_RANGES), np.int64)
    uniq_per_cw = [[None] * N_WIN for _ in range(NCORES)]
    for c in range(NCORES):
        cw = cores[c]
        gwin = cw["key"] // R
        for wv in range(N_WIN):
            uu = np.unique(cw["s"][gwin == wv])    # sorted ascending
            uniq_per_cw[c][wv] = uu
            seg_cnt[c, wv] = np.bincount(uu // RANGE_W, minlength=SRC_RANGES)
    CS = _round_up(seg_cnt.max(axis=0), P)         # [N_WIN, SRC_RANGES] shared
    seg_base = np.zeros((N_WIN, SRC_RANGES + 1), np.int64)
    for wv in range(N_WIN):
        seg_base[wv, 1:] = np.cumsum(CS[wv])
    CT = seg_base[:, -1].astype(np.int64)          # compact tokens per window

    # ---- per-core gather index lists ----
    gidx_cols = []   # compact-table DRAM gather images, per core
    eidx_cols = []   # edge (SBUF-source) gather images, per core
    midx_cols = []   # pass-2 msg gather images, per core
    tgtw_arrs = []   # pass-2 [chunks, 128, 2] f32 meta, per core

    # shared pass-2 structure from target-block counts
    n_tb = np.zeros((NCORES, TBLOCKS), np.int64)
    for c in range(NCORES):
        n_tb[c] = np.bincount(cores[c]["tloc"] // P, minlength=TBLOCKS)
    cap_tb = n_tb.max(axis=0)
    chunks_tb = _ceil(cap_tb, P)                   # chunks per tblock (may be 0)
    slots_tb = chunks_tb * P
    # per-window chunk/slot totals
    win_chunks = [int(chunks_tb[WIN_TB_START[wv]:WIN_TB_START[wv + 1]].sum())
                  for wv in range(N_WIN)]
    win_slots = [ch * P for ch in win_chunks]

    # edge gather list length per window (shared): groups padded to caps,
    # then padded to a multiple of 128 for gather ops
    EWpad = [_round_up(EW[wv], P) if EW[wv] else 0 for wv in range(N_WIN)]

    for c in range(NCORES):
        cw = cores[c]
        gwin = cw["key"] // R
        g_img, e_img, m_img = [], [], []
        tg_meta = []
        for wv in range(N_WIN):
            uu = uniq_per_cw[c][wv]
            # compact gather lists (4 ranges, padded to CS with 0)
            for g in range(SRC_RANGES):
                if CS[wv][g] == 0:
                    continue
                seg = uu[(uu >= g * RANGE_W) & (uu < (g + 1) * RANGE_W)] - g * RANGE_W
                pad = np.zeros(CS[wv][g], np.int64)
                pad[:len(seg)] = seg
                g_img.append(pad)
            # token id of each source node in the window's compact table
            lut = np.zeros(N_NODES, np.int64)
            for g in range(SRC_RANGES):
                seg = uu[(uu >= g * RANGE_W) & (uu < (g + 1) * RANGE_W)]
                lut[seg] = seg_base[wv][g] + np.arange(len(seg))
            # edge gather list: per group, edges then pad-to-cap with token 0
            wmask = gwin == wv
            ws = cw["s"][wmask]
            wr = cw["r"][wmask]
            wt = cw["tloc"][wmask]
            ww = cw["w"][wmask]
            elist = np.zeros(EWpad[wv], np.int64)
            # group-relative placement & msg position of each real edge
            pos_in_win = np.zeros(len(ws), np.int64)
            ecursor = 0
            for r in range(R):
                gsel = wr == r
                cnt = int(gsel.sum())
                if cnt:
                    elist[ecursor:ecursor + cnt] = lut[ws[gsel]]
                    pos_in_win[gsel] = ecursor + np.arange(cnt)
                ecursor += int(cap[wv * R + r])
            e_img.append(elist)
            # pass-2: sort window's real edges by target
            o2 = np.argsort(wt, kind="stable")
            wt2, ww2, pos2 = wt[o2], ww[o2], pos_in_win[o2]
            tb2 = wt2 // P
            mlist = np.zeros(win_slots[wv], np.int64)
            meta = np.zeros((win_chunks[wv], P, 2), np.float32)
            scursor = 0
            for tbi in range(WIN_TB_START[wv], WIN_TB_START[wv + 1]):
                if chunks_tb[tbi] == 0:
                    continue
                sel = tb2 == tbi
                cnt = int(sel.sum())
                nslots = int(slots_tb[tbi])
                mlist[scursor:scursor + cnt] = pos2[sel]
                mview = meta.reshape(-1, 2)
                mview[scursor:scursor + cnt, 0] = (wt2[sel] - tbi * P).astype(np.float32)
                mview[scursor:scursor + cnt, 1] = ww2[sel]
                # pad slots: idx 0, tgt 0, w 0 (already zeros)
                scursor += nslots
            m_img.append(mlist)
            tg_meta.append(meta)
        gidx_cols.append(_idx_image(np.concatenate(g_img)) if g_img else
                         np.zeros((128, 8), np.int16))
        eidx_cols.append(_idx_image(np.concatenate(e_img)))
        midx_cols.append(_idx_image(np.concatenate(m_img)))
        tgtw_arrs.append(np.concatenate(tg_meta, axis=0))

    # ---- shared build structure ----
    # pass-1 chunks: (slab_index, slab_col, n, rel, msg_off) per window
    p1 = [[] for _ in range(N_WIN)]
    eop_sizes = [[] for _ in range(N_WIN)]      # edge gather op sizes
    for wv in range(N_WIN):
        rem = EWpad[wv]
        while rem > 0:
            t = min(GOP, rem)
            eop_sizes[wv].append(t)
            rem -= t
        for r in range(R):
            g = wv * R + r
            cgap = int(cap[g])
            gstart = int(grp_off[g] - MO[wv])    # window-local edge index
            done = 0
            while done < cgap:
                sl = (gstart + done) // GOP
                scol = (gstart + done) % GOP
                n = min(cgap - done, P, GOP - scol,
                        eop_sizes[wv][sl] - scol if sl < len(eop_sizes[wv]) else P)
                p1[wv].append((sl, scol, int(n), r,
                               int(grp_off[g] + done)))
                done += n
    mop_sizes = [[] for _ in range(N_WIN)]
    for wv in range(N_WIN):
        rem = win_slots[wv]
        while rem > 0:
            t = min(GOP, rem)
            mop_sizes[wv].append(t)
            rem -= t

    shared = {
        "CS": CS, "CT": CT, "EWpad": EWpad, "MO": MO,
        "total_msgs": total_msgs,
        "chunks_tb": chunks_tb, "win_chunks": win_chunks,
        "win_slots": win_slots,
        "p1": p1, "eop_sizes": eop_sizes, "mop_sizes": mop_sizes,
        "gidx_w": gidx_cols[0].shape[1],
        "eidx_w": eidx_cols[0].shape[1],
        "midx_w": midx_cols[0].shape[1],
        "tgtw_n": tgtw_arrs[0].shape[0],
    }

    # ---- device input arrays ----
    xbf = np.asarray(x, dtype=np.float32).astype(bf16)
    wd = np.zeros((P, (R + 1) * P), dtype=bf16)   # dense blockdiag, (i, r*128+j)
    blk = np.asarray(blocks, dtype=np.float32)
    for r in range(R):
        for b in range(NB):
            wd[b * BS:(b + 1) * BS, r * P + b * BS:r * P + (b + 1) * BS] = \
                blk[r, b].astype(bf16)
    iota = np.broadcast_to(np.arange(P, dtype=np.float32), (P, P)).copy()

    in_maps = []
    for c in range(NCORES):
        in_maps.append({
            "xbf": xbf,
            "wd": wd,
            "iota": iota,
            "gidx": gidx_cols[c],
            "eidx": eidx_cols[c],
            "midx": midx_cols[c],
            "tgtw": tgtw_arrs[c],
        })
    return shared, in_maps


def _build_nc(shared):
    _patch_tile_drain()
    import concourse.bacc as bacc
    import concourse.mybir as mybir
    from concourse.tile import TileContext

    CS, CT = shared["CS"], shared["CT"]
    EWpad, MO = shared["EWpad"], shared["MO"]
    chunks_tb = shared["chunks_tb"]
    win_chunks = shared["win_chunks"]
    p1, eop_sizes, mop_sizes = shared["p1"], shared["eop_sizes"], shared["mop_sizes"]

    f32 = mybir.dt.float32
    bf = mybir.dt.bfloat16
    i16 = mybir.dt.int16

    nc = bacc.Bacc("TRN2", target_bir_lowering=False, debug=False)
    xbf_d = nc.dram_tensor("xbf", [N_NODES, D], bf, kind="ExternalInput")
    wd_d = nc.dram_tensor("wd", [P, (R + 1) * P], bf, kind="ExternalInput")
    iota_d = nc.dram_tensor("iota", [P, P], f32, kind="ExternalInput")
    gidx_d = nc.dram_tensor("gidx", [P, shared["gidx_w"]], i16, kind="ExternalInput")
    eidx_d = nc.dram_tensor("eidx", [P, shared["eidx_w"]], i16, kind="ExternalInput")
    midx_d = nc.dram_tensor("midx", [P, shared["midx_w"]], i16, kind="ExternalInput")
    tgtw_d = nc.dram_tensor("tgtw", [shared["tgtw_n"], P, 2], f32, kind="ExternalInput")
    out_d = nc.dram_tensor("out", [NT_PAD, D], f32, kind="ExternalOutput")
    msgs_d = nc.dram_tensor("msgs", [max(shared["total_msgs"], P), D], bf,
                            kind="Internal")

    max_ct = int(max(CT)) if len(CT) else P
    max_meta = max(max(win_chunks), 1) * 2

    with TileContext(nc) as tc:
        with (
            tc.tile_pool(name="cpool", bufs=1) as cpool,
            tc.tile_pool(name="ctab", bufs=2) as ctab,
            tc.tile_pool(name="idxp", bufs=4) as idxp,
            tc.tile_pool(name="xtp", bufs=2) as xtp,
            tc.tile_pool(name="msp", bufs=2) as msp,
            tc.tile_pool(name="p1s", bufs=4) as p1s,
            tc.tile_pool(name="metap", bufs=2) as metap,
            tc.tile_pool(name="spool", bufs=4) as spool,
            tc.tile_pool(name="outp", bufs=3) as outp,
            tc.tile_pool(name="psA", bufs=2, space="PSUM") as psA,
            tc.tile_pool(name="psB", bufs=2, space="PSUM") as psB,
        ):
            wd_t = cpool.tile([P, (R + 1) * P], bf)
            iota_t = cpool.tile([P, P], f32)
            nc.sync.dma_start(out=wd_t[:], in_=wd_d[:])
            nc.sync.dma_start(out=iota_t[:], in_=iota_d[:])

            gcol = ecol = mcol = 0
            chbase = 0
            for wv in range(N_WIN):
                if EWpad[wv] == 0:
                    continue
                # ---- compact source table ----
                ct = ctab.tile([P, max_ct], bf, tag="ct")
                col = 0
                for g in range(SRC_RANGES):
                    cs = int(CS[wv][g])
                    if cs == 0:
                        continue
                    it = idxp.tile([P, cs // 16], i16, tag="gi")
                    nc.sync.dma_start(out=it[:], in_=gidx_d[:, gcol:gcol + cs // 16])
                    nc.gpsimd.dma_gather(
                        out_ap=ct[:, col:col + cs].rearrange("p (c e) -> p c e", e=P),
                        in_ap=xbf_d[g * RANGE_W:, :],
                        idxs_ap=it[:],
                        num_idxs=cs, num_idxs_reg=cs, elem_size=D,
                        single_packet=False,
                    )
                    gcol += cs // 16
                    col += cs

                # ---- edge gather -> xT slabs ----
                slabs = []
                for opn in eop_sizes[wv]:
                    it = idxp.tile([P, opn // 16], i16, tag="ei")
                    nc.sync.dma_start(out=it[:], in_=eidx_d[:, ecol:ecol + opn // 16])
                    xts = xtp.tile([P, GOP], bf, tag="xts")
                    nc.gpsimd.dma_gather(
                        out_ap=xts[:, :opn].rearrange("p (c e) -> p c e", e=opn),
                        in_ap=ct[:, :int(CT[wv])],
                        idxs_ap=it[:],
                        num_idxs=opn, num_idxs_reg=opn, elem_size=D,
                        single_packet=False,
                        transpose=True,
                        sbuf_tokens_per_rank=P,
                        sbuf_free_dim_per_rank=D * 2,
                    )
                    ecol += opn // 16
                    slabs.append(xts)

                # ---- pass 1: message chunks ----
                for (sl, scol, n, r, moff) in p1[wv]:
                    mp = psA.tile([P, P], f32, tag="mp")
                    nc.tensor.matmul(
                        out=mp[:n, :],
                        lhsT=slabs[sl][:, scol:scol + n],
                        rhs=wd_t[:, r * P:(r + 1) * P],
                        start=True, stop=True,
                    )
                    ms = p1s.tile([P, P], bf, tag="ms")
                    nc.vector.tensor_copy(out=ms[:n, :], in_=mp[:n, :])
                    nc.sync.dma_start(out=msgs_d[moff:moff + n, :], in_=ms[:n, :])

                # ---- pass 2 ----
                wch = win_chunks[wv]
                if wch == 0:
                    continue
                meta_t = metap.tile([P, max_meta], f32, tag="meta")
                nc.sync.dma_start(
                    out=meta_t[:, :wch * 2].rearrange("p (c k) -> p c k", k=2),
                    in_=tgtw_d[chbase:chbase + wch].rearrange("c p k -> p c k"),
                )
                mslabs = []
                for opn in mop_sizes[wv]:
                    it = idxp.tile([P, opn // 16], i16, tag="mi")
                    nc.sync.dma_start(out=it[:], in_=midx_d[:, mcol:mcol + opn // 16])
                    msl = msp.tile([P, GOP], bf, tag="mslab")
                    nc.gpsimd.dma_gather(
                        out_ap=msl[:, :opn].rearrange("p (c e) -> p c e", e=D),
                        in_ap=msgs_d[MO[wv]:, :],
                        idxs_ap=it[:],
                        num_idxs=opn, num_idxs_reg=opn, elem_size=D,
                        single_packet=False,
                    )
                    mcol += opn // 16
                    mslabs.append(msl)

                ci = 0  # window-local chunk counter
                for tbi in range(WIN_TB_START[wv], WIN_TB_START[wv + 1]):
                    K = int(chunks_tb[tbi])
                    if K == 0:
                        continue
                    acc = psB.tile([P, P], f32, tag="acc")
                    for k in range(K):
                        gpos = ci * P
                        sl, scol = gpos // GOP, gpos % GOP
                        S_t = spool.tile([P, P], bf, tag="S")
                        nc.vector.tensor_scalar(
                            out=S_t[:],
                            in0=iota_t[:],
                            scalar1=meta_t[:, 2 * ci:2 * ci + 1],
                            scalar2=meta_t[:, 2 * ci + 1:2 * ci + 2],
                            op0=mybir.AluOpType.is_equal,
                            op1=mybir.AluOpType.mult,
                        )
                        nc.tensor.matmul(
                            out=acc[:],
                            lhsT=S_t[:],
                            rhs=mslabs[sl][:, scol:scol + P],
                            start=(k == 0), stop=(k == K - 1),
                        )
                        ci += 1
                    ot = outp.tile([P, P], f32, tag="ot")
                    nc.vector.tensor_copy(out=ot[:], in_=acc[:])
                    nc.sync.dma_start(out=out_d[tbi * P:(tbi + 1) * P, :], in_=ot[:])
                chbase += wch
    nc.finalize()
    return nc


def kernel(x, blocks, edge_weights, source, target, edge_type):
    from concourse import bass_utils

    shared, in_maps = _preprocess(x, blocks, edge_weights, source, target,
                                  edge_type)
    nc = _build_nc(shared)
    res = bass_utils.run_bass_kernel_spmd(
        nc, in_maps, core_ids=list(range(NCORES)), trace=TRACE,
    )
    out = np.concatenate([res.results[c]["out"][:NT] for c in range(NCORES)],
                         axis=0)
    if TRACE:
        kernel.last_exec_ns = res.exec_time_ns
        kernel.last_result = res
    return out.astype(np.float32)


kernel.last_exec_ns = None

